# revision 2
# baseline (speedup 1.0000x reference)
"""Trainium2 Bass kernel: batched controlled-system dynamics (N = 2^20 states).

Strategy (v2):
  - Pure data parallel over 8 NeuronCores: contiguous slices of the batch axis.
  - Everything elementwise in a ribbon layout [128, 1024] per core, fp16
    tensors (DVE 2x mode), two column-chunks for load/compute/store overlap.
  - Host packs ALL inputs into one fp16 array [NCH, 128, 6, C]
    (x1, v1, x2, v2, A*xc, t) so each chunk is ONE load DMA with 6KB
    partition lines.  Outputs dv1/dv2/dxc are packed into one fp16 store per
    chunk; dx1/dx2 are direct fp16 copies of the v1/v2 input views.  Host
    unpacks + upcasts to fp32 (cuts store traffic in half).
  - The friction MLP (1 -> 64 -> 2, tanh + softplus heads) collapses to
    1D functions of v2 (b1 == 0 makes g0 odd):
      kinetic(v2)  = softplus(lead * G + eb),  G = v2 * prod_j f_j(w)
    where w = affine(v2^2) and f_j are monic quadratic/linear factors of the
    Chebyshev-fit polynomial (evaluated in fp16 on DVE; `lead` folds into
    the Exp activation's input scale).  Stiction limit is linearized around
    0 (exact to ~1e-4 on its +-0.01 window).
  - Engine split: Act does sin/square/sign/exp/ln (+ W^2), Pool (gpsimd)
    does mask/stiction-bounds/x-differences, DVE does the rest.
"""

import numpy as np

# physical system constants (match the reference)
M1, M2 = 1.0, 1.5
K1, K2 = 2.0, 3.0
C1, C2 = 0.5, 0.8
KARNOPP_DV = 0.01
REF_AMP, REF_OMEGA = 0.5, 0.5

N_CORES = 8
N_TOTAL = 1 << 20
N_CORE = N_TOTAL // N_CORES    # 131072
P = 128
F = N_CORE // P                # 1024 ribbon columns
NCH = 2                        # compute/DMA chunks
C = F // NCH                   # 512 columns per chunk
NROW_IN = 6                    # x1 v1 x2 v2 xc' t
NROW_OUT = 3                   # dv1 dv2 dxc

_compile_cache = {}


def _softplus(x):
    return np.log1p(np.exp(-np.abs(x))) + np.maximum(x, 0.0)


def _fit_friction(W1, b1, W2, b2, v2, vmax):
    """Collapse the friction MLP to 1D functions of v2 and pick the cheapest
    factored-polynomial representation whose *exact, data-weighted* L2 error
    stays within budget."""
    W1 = W1.astype(np.float64).reshape(-1)      # [H]
    b1 = b1.astype(np.float64).reshape(-1)
    W2 = W2.astype(np.float64)                  # [H, 2]
    b2 = b2.astype(np.float64).reshape(-1)

    def gg(v, col):
        return np.tanh(np.outer(v, W1) + b1) @ W2[:, col]

    umax = vmax * vmax
    su = 2.0 / umax
    bu = -1.0

    M = 4000
    wn = np.cos(np.pi * (np.arange(M) + 0.5) / M)
    u = (wn + 1.0) / 2.0 * umax
    v = np.sqrt(np.maximum(u, 1e-12))
    gp = gg(v, 0)
    gm = gg(-v, 0)
    E = (gp + gm) / 2.0          # even part of g0 (== 0 when b1 == 0)
    O = (gp - gm) / 2.0 / v      # odd part / v, a function of u (hence w)

    eb = b2[0] + float(np.mean(E))
    weight = v + 0.02

    # stiction limit, linearized at 0:  L(v) = softplus(g1(v) + b2[1])
    d = 1e-4
    g1p = (gg(np.array([d]), 1)[0] - gg(np.array([-d]), 1)[0]) / (2 * d)
    g10 = gg(np.array([0.0]), 1)[0] + b2[1]
    L0 = _softplus(g10)
    sig = 1.0 / (1.0 + np.exp(-g10))
    L1 = sig * g1p

    # exact reference friction on the actual v2 batch (float64)
    av = np.abs(v2).astype(np.float64)
    vv = v2.astype(np.float64)
    kin_exact = _softplus(gg(vv, 0) + b2[0])
    kinetic_term = kin_exact * np.sign(vv)       # kinetic branch: KIN*sgn
    is_kin = av >= KARNOPP_DV

    # reference-magnitude proxy for the global L2 denominator: use dv2 scale
    # times a conservative share (dv2 errors are what the fit perturbs).
    import numpy.polynomial.chebyshev as Cc
    import numpy.polynomial.polynomial as Pp

    wv = (vv * vv) * su + bu
    best = None
    for deg in range(3, 13):
        cc = Cc.chebfit(wn, O, deg, w=weight)
        mono = Cc.cheb2poly(cc)
        approx = vv * Pp.polyval(wv, mono)       # ~ g0(v2)
        kin_fit = _softplus(approx + eb)
        err = np.abs((kin_fit - kin_exact)[is_kin])
        l2 = float(np.sqrt(np.sum(err * err)))   # abs L2 over kinetic elems
        # budget: contribution to global rel err.  ||expected|| ~ 13000 for
        # this problem size; keep fit-only contribution under ~2.5e-3 of it.
        if l2 / 13000.0 < 2.5e-3 or deg == 12:
            best = (mono, deg, l2)
            break
    mono, deg, l2 = best

    # --- factor the monomial polynomial into monic quadratics/linears ---
    lead = mono[-1]
    roots = np.roots(mono[::-1])
    quads = []   # (b, c): w^2 + b*w + c
    lins = []    # r: w - r
    used = np.zeros(len(roots), bool)
    for i, r in enumerate(roots):
        if used[i]:
            continue
        if abs(r.imag) > 1e-12:
            for j in range(i + 1, len(roots)):
                if not used[j] and abs(roots[j] - r.conjugate()) < 1e-8:
                    used[j] = True
                    break
            quads.append((-2.0 * r.real, r.real ** 2 + r.imag ** 2))
            used[i] = True
        else:
            used[i] = True
            lins.append(r.real)
    # pair real roots into quadratics (adjacent after sorting)
    lins.sort()
    while len(lins) >= 2:
        r1 = lins.pop(0)
        r2 = lins.pop(-1)
        quads.append((-(r1 + r2), r1 * r2))
    lin = lins[0] if lins else None

    # overflow guard for the fp16 product chain: rescale w -> w/s
    s = 1.0
    for _ in range(40):
        wg = np.linspace(-1.0, 1.0, 2001) / s
        prod = np.ones_like(wg)
        mx = 0.0
        for (qb, qc) in quads:
            fv = wg * wg + (qb / s) * wg + qc / (s * s)
            mx = max(mx, np.abs(fv).max())
            prod = prod * fv
            mx = max(mx, np.abs(prod).max())
        if lin is not None:
            prod = prod * (wg - lin / s)
            mx = max(mx, np.abs(prod).max())
        if mx * vmax < 2.0e4:
            break
        s *= 2.0
    c_scale = lead * (s ** (len(quads) * 2 + (1 if lin is not None else 0)))

    return dict(
        su=su / s, bu=bu / s,
        quads=[(qb / s, qc / (s * s)) for (qb, qc) in quads],
        lin=(lin / s if lin is not None else None),
        c_scale=float(c_scale), eb=float(eb),
        L0=float(L0), L1=float(L1), deg=deg, fit_l2=l2,
    )


def _build_program(consts):
    """Build the SPMD Bass program (same on all 8 cores)."""
    import concourse.bacc as bacc
    import concourse.mybir as mybir
    import bass_rust as _bass_rust
    from concourse import tile
    from concourse.tile_rust import add_dep_helper
    from concourse.hw_specs import get_activation_tables

    fp32 = mybir.dt.float32
    fp16 = mybir.dt.float16
    u8 = mybir.dt.uint8
    Alu = mybir.AluOpType
    Act = mybir.ActivationFunctionType

    class _Bacc(bacc.Bacc):
        # Force Exp and Ln to resolve to the combined natural_log_exp_and_others
        # table so the whole kernel needs only two table loads (trig + exp/ln).
        def insert_act_table_loads(self):
            has_activation = any(
                isinstance(i, mybir.InstActivation)
                for b in self.main_func.blocks
                for i in b.instructions
            )
            if not has_activation:
                return
            tables = list(get_activation_tables(self.m.arch).items())
            fixed = []
            for name, funcs in tables:
                if name != "trig_and_small":
                    funcs = funcs - {Act.Sign, Act.Identity, Act.Sin}
                if name != "natural_log_exp_and_others":
                    funcs = funcs - {Act.Exp, Act.Ln}
                fixed.append((name, funcs))
            _bass_rust.insert_act_table_loads(self, fixed)

    c = consts
    neg_pi = float(np.float32(-np.pi))
    nc = _Bacc()

    def reg_const(val):
        v = float(val)
        if (fp32, v) not in nc.const_aps.aps:
            tsr = nc.alloc_sbuf_tensor(
                f"constu-f32-{len(nc.const_aps.aps)}", [128, 1], fp32)
            nc.gpsimd.memset(tsr.ap(), v)
            nc.const_aps.aps[(fp32, v)] = tsr.ap()

    for v in (0.0, neg_pi, c["eb"], 1.0):
        reg_const(v)
    nc.all_engine_barrier()

    zin_d = nc.dram_tensor("zin", [NCH, P, NROW_IN * C], fp16,
                           kind="ExternalInput")
    om_d = nc.dram_tensor("om", [NCH, P, NROW_OUT * C], fp16,
                          kind="ExternalOutput")
    ox1_d = nc.dram_tensor("ox1", [N_CORE], fp16, kind="ExternalOutput")
    ox2_d = nc.dram_tensor("ox2", [N_CORE], fp16, kind="ExternalOutput")

    quads = c["quads"]
    lin = c["lin"]

    with tile.TileContext(nc) as tc:
        with tc.tile_pool(name="sb", bufs=1) as pool:
            act_instrs = [[], []]   # [trig-phase, expln-phase]

            def tl(tag, dt=fp16, cols=C):
                return pool.tile([P, cols], dt, tag=tag, name=tag)

            for ch in range(NCH):
                sfx = f"_{ch}"
                L = tl("L" + sfx, cols=NROW_IN * C)
                nc.sync.dma_start(out=L[:], in_=zin_d[ch])
                X1 = L[:, 0 * C:1 * C]; V1 = L[:, 1 * C:2 * C]
                X2 = L[:, 2 * C:3 * C]; V2 = L[:, 3 * C:4 * C]
                XCA = L[:, 4 * C:5 * C]; TH = L[:, 5 * C:6 * C]

                # early pass-through stores (only need the load)
                rb = F  # full-core ribbon column count
                nc.sync.dma_start(
                    out=ox1_d[:].rearrange("(p i) -> p i", p=P)[:, ch * C:(ch + 1) * C],
                    in_=V1)
                nc.sync.dma_start(
                    out=ox2_d[:].rearrange("(p i) -> p i", p=P)[:, ch * C:(ch + 1) * C],
                    in_=V2)

                # ---- Act phase 1 (trig_and_small): S, U, SGN ----
                S = tl("S" + sfx)
                i1 = nc.scalar.activation(S[:], TH, Act.Sin, bias=neg_pi, scale=0.5)
                U = tl("U" + sfx)
                i2 = nc.scalar.activation(U[:], V2, Act.Square)
                SGN = tl("SGN" + sfx)
                i3 = nc.scalar.activation(SGN[:], V2, Act.Sign)
                act_instrs[0] += [i1, i2, i3]

                # ---- Pool: mask, stiction bounds, x/v differences ----
                MASK = pool.tile([P, C], u8, tag="MASK" + sfx, name="MASK" + sfx)
                nc.gpsimd.tensor_single_scalar(MASK[:], U[:], KARNOPP_DV ** 2,
                                               Alu.is_lt)
                LP = tl("LP" + sfx)
                nc.gpsimd.tensor_scalar(LP[:], V2, c["L1"], c["L0"],
                                        Alu.mult, Alu.add)
                NLP = tl("NLP" + sfx)
                nc.gpsimd.tensor_single_scalar(NLP[:], LP[:], -1.0, Alu.mult)
                HA = tl("HA" + sfx)
                nc.gpsimd.tensor_tensor(HA[:], X1, X2, Alu.subtract)
                HB = tl("HB" + sfx)
                nc.gpsimd.tensor_tensor(HB[:], V1, V2, Alu.subtract)

                # ---- DVE: polynomial for g0 (factored form, fp16) ----
                W = tl("W" + sfx)
                nc.vector.tensor_scalar(W[:], U[:], c["su"], c["bu"],
                                        Alu.mult, Alu.add)
                W2T = tl("W2T" + sfx)
                i4 = nc.scalar.activation(W2T[:], W[:], Act.Square)
                PACC = None
                for qi, (qb, qc) in enumerate(quads):
                    R = tl(f"R{qi}" + sfx)
                    nc.vector.tensor_scalar(R[:], W[:], qb, qc, Alu.mult, Alu.add)
                    FQ = tl(f"FQ{qi}" + sfx)
                    nc.vector.tensor_tensor(FQ[:], W2T[:], R[:], Alu.add)
                    if PACC is None:
                        PACC = FQ
                    else:
                        NP_ = tl(f"PP{qi}" + sfx)
                        nc.vector.tensor_tensor(NP_[:], PACC[:], FQ[:], Alu.mult)
                        PACC = NP_
                if lin is not None:
                    FL = tl("FL" + sfx)
                    nc.vector.tensor_scalar(FL[:], W[:], 1.0, -lin,
                                            Alu.mult, Alu.add)
                    NP_ = tl("PPL" + sfx)
                    nc.vector.tensor_tensor(NP_[:], PACC[:], FL[:], Alu.mult)
                    PACC = NP_
                G = tl("G" + sfx)
                nc.vector.tensor_tensor(G[:], PACC[:], V2, Alu.mult)

                # ---- Act phase 2 (natural_log_exp): Q, KIN ----
                Q = tl("Q" + sfx, dt=fp32)
                i5 = nc.scalar.activation(Q[:], G[:], Act.Exp,
                                          bias=c["eb"], scale=c["c_scale"])
                KIN = tl("KIN" + sfx)
                i6 = nc.scalar.activation(KIN[:], Q[:], Act.Ln, bias=1.0)
                act_instrs[1] += [i4, i5, i6]

                # ---- DVE: friction select + dv2 ----
                PSI = tl("PSI" + sfx)   # KIN * sgn(v2)  (force units)
                nc.vector.tensor_tensor(PSI[:], KIN[:], SGN[:], Alu.mult)
                B4 = tl("B4" + sfx)     # ha + (C2/K2) hb = F_net/K2
                nc.vector.scalar_tensor_tensor(B4[:], HB[:], C2 / K2, HA[:],
                                               Alu.mult, Alu.add)
                FN = tl("FN" + sfx)     # F_net
                nc.vector.tensor_single_scalar(FN[:], B4[:], K2, Alu.mult)
                MX = tl("MX" + sfx)
                nc.vector.tensor_tensor(MX[:], FN[:], NLP[:], Alu.max)
                MM = tl("MM" + sfx)
                nc.vector.tensor_tensor(MM[:], MX[:], LP[:], Alu.min)
                nc.vector.copy_predicated(PSI[:], MASK[:], MM[:])

                OUT = tl("OUT" + sfx, cols=NROW_OUT * C)
                ODV1 = OUT[:, 0 * C:1 * C]
                ODV2 = OUT[:, 1 * C:2 * C]
                ODXC = OUT[:, 2 * C:3 * C]

                DS = tl("DS" + sfx)
                nc.vector.tensor_tensor(DS[:], FN[:], PSI[:], Alu.subtract)
                nc.vector.tensor_single_scalar(ODV2, DS[:], 1.0 / M2, Alu.mult)

                # ---- DVE: e, dv1, d_xc ----
                SH = tl("SH" + sfx)     # 0.5 sin(0.5 t)   (S = -sin(0.5t))
                nc.vector.tensor_single_scalar(SH[:], S[:], -0.5, Alu.mult)
                E = tl("E" + sfx)       # x2_ref - x2
                nc.vector.tensor_tensor(E[:], SH[:], X2, Alu.subtract)
                B1 = tl("B1" + sfx)     # K*e + A*xc
                nc.vector.scalar_tensor_tensor(B1[:], E[:], c["K"], XCA,
                                               Alu.mult, Alu.add)
                B2 = tl("B2" + sfx)     # x1 + (C1/K1) v1
                nc.vector.scalar_tensor_tensor(B2[:], V1, C1 / K1, X1,
                                               Alu.mult, Alu.add)
                B3 = tl("B3" + sfx)     # B1 - K1*B2
                nc.vector.scalar_tensor_tensor(B3[:], B2[:], -K1, B1[:],
                                               Alu.mult, Alu.add)
                nc.vector.tensor_tensor(ODV1, B3[:], FN[:], Alu.subtract)
                # d_xc = e - p*xc = e - (p/A) * (A*xc)
                nc.vector.scalar_tensor_tensor(ODXC, XCA, -c["p"] / c["A"],
                                               E[:], Alu.mult, Alu.add)

                nc.sync.dma_start(out=om_d[ch], in_=OUT[:])

            # keep the Act engine's table phases coherent: chain nosync deps
            seq = act_instrs[0] + act_instrs[1]
            for a, b in zip(seq, seq[1:]):
                add_dep_helper(b.ins, a.ins, sync=False, reason="act table order")

    nc.finalize()
    return nc


def _prepare(inputs):
    """Host-side constant folding + program build (cached on weight values)."""
    logK = np.float32(inputs["logK"]); logz = np.float32(inputs["logz"])
    logp = np.float32(inputs["logp"])
    W1 = np.asarray(inputs["W1"], dtype=np.float32)
    b1 = np.asarray(inputs["b1"], dtype=np.float32)
    W2 = np.asarray(inputs["W2"], dtype=np.float32)
    b2 = np.asarray(inputs["b2"], dtype=np.float32)
    v2 = np.asarray(inputs["z"][3], dtype=np.float32)
    vmax = float(np.abs(v2).max()) * 1.02 + 1e-3

    key = (logK.tobytes(), logz.tobytes(), logp.tobytes(), W1.tobytes(),
           b1.tobytes(), W2.tobytes(), b2.tobytes(), round(vmax, 3))
    if key in _compile_cache:
        return _compile_cache[key]

    K = np.float32(np.exp(logK))
    z_ctrl = np.float32(np.exp(logz))
    p_ctrl = np.float32(np.exp(logp))
    A = np.float32(K * (z_ctrl - p_ctrl))

    fit = _fit_friction(W1, b1, W2, b2, v2, vmax)

    consts = dict(
        K=float(K), p=float(p_ctrl), A=float(A),
        su=fit["su"], bu=fit["bu"], quads=fit["quads"], lin=fit["lin"],
        c_scale=fit["c_scale"], eb=fit["eb"],
        L0=fit["L0"], L1=fit["L1"],
    )
    nc = _build_program(consts)
    _compile_cache[key] = (nc, fit, consts)
    return nc, fit, consts


def _run(inputs, trace=False):
    from concourse.bass_utils import run_bass_kernel_spmd

    nc, _fit, consts = _prepare(inputs)

    t = np.ascontiguousarray(np.asarray(inputs["t"], dtype=np.float32))
    z = np.ascontiguousarray(np.asarray(inputs["z"], dtype=np.float32))

    # pack [6, N]: x1, v1, x2, v2, A*xc, t  -> fp16 [cores, NCH, P, 6, C]
    rows = np.empty((NROW_IN, N_TOTAL), dtype=np.float16)
    rows[0] = z[0]; rows[1] = z[1]; rows[2] = z[2]; rows[3] = z[3]
    rows[4] = (z[4].astype(np.float64) * consts["A"]).astype(np.float16)
    rows[5] = t
    pk = rows.reshape(NROW_IN, N_CORES, P, NCH, C).transpose(1, 3, 2, 0, 4)
    pk = np.ascontiguousarray(pk)  # [cores, NCH, P, 6, C]

    in_maps = [{"zin": pk[i].reshape(NCH, P, NROW_IN * C)} for i in range(N_CORES)]

    res = run_bass_kernel_spmd(nc, in_maps, core_ids=list(range(N_CORES)),
                               trace=trace)
    out = np.empty((5, N_TOTAL), dtype=np.float32)
    for i in range(N_CORES):
        sl = slice(i * N_CORE, (i + 1) * N_CORE)
        om = res.results[i]["om"].reshape(NCH, P, NROW_OUT, C)
        om = om.transpose(2, 1, 0, 3).reshape(NROW_OUT, N_CORE)
        out[0, sl] = res.results[i]["ox1"].reshape(N_CORE)
        out[1, sl] = om[0]
        out[2, sl] = res.results[i]["ox2"].reshape(N_CORE)
        out[3, sl] = om[1]
        out[4, sl] = om[2]
    return out, res


def kernel(**inputs):
    out, _res = _run(inputs, trace=False)
    return out


# revision 3
# speedup vs baseline: 2.2706x; 2.2706x over previous
"""Trainium2 Bass kernel: batched controlled-system dynamics (N = 2^20 states).

Strategy (v3):
  - Pure data parallel over 8 NeuronCores: contiguous slices of the batch axis.
  - Everything elementwise in a ribbon layout [128, 1024] per core, fp16
    tensors, two column-chunks for load/compute/store overlap.
  - Host packs ALL inputs into one fp16 array [NCH, 128, 6, C]
    (x1, v1, x2, v2, A*xc, t): ONE load DMA per chunk, 6KB partition lines.
    Outputs are stored per row in fp16 (dx1/dx2 ship straight from the input
    views); host unpacks + upcasts to fp32.  Total HBM traffic ~2.75MB/core.
  - Friction MLP (1 -> 64 -> 2, tanh + softplus heads) collapses to 1D
    functions of v2 (b1 == 0 makes g0 odd):
      kinetic = softplus(lead * G + eb),  G = v2 * prod_j f_j(u'), u' = s*v2^2
    with monic quadratic/linear factors f_j of the Chebyshev fit, evaluated
    in fp16 on DVE; `lead` folds into the Exp activation's input scale and
    the overflow-guard rescale s folds into the Square activation's input
    scale.  Stiction limit uses a CONSTANT bound L0 (the |v2| < 0.01 window
    makes the L1*v2 term < 1e-2*L1 — negligible).
  - Engine split: Act does sin/square(x2)/sign/exp/ln, DVE does everything
    else.  The Pool engine is never used for compute (its single-scalar ops
    take ~7.5us on HW).
"""

import numpy as np

# physical system constants (match the reference)
M1, M2 = 1.0, 1.5
K1, K2 = 2.0, 3.0
C1, C2 = 0.5, 0.8
KARNOPP_DV = 0.01
REF_AMP, REF_OMEGA = 0.5, 0.5

N_CORES = 8
N_TOTAL = 1 << 20
N_CORE = N_TOTAL // N_CORES    # 131072
P = 128
F = N_CORE // P                # 1024 ribbon columns
NCH = 2                        # compute/DMA chunks
C = F // NCH                   # 512 columns per chunk
NROW_IN = 6                    # x1 v1 x2 v2 xc' t

_compile_cache = {}


def _softplus(x):
    return np.log1p(np.exp(-np.abs(x))) + np.maximum(x, 0.0)


def _fit_friction(W1, b1, W2, b2, v2, vmax):
    """Collapse the friction MLP to 1D functions of v2 and pick the cheapest
    factored-polynomial representation whose exact, data-weighted L2 error
    stays within budget."""
    W1 = W1.astype(np.float64).reshape(-1)      # [H]
    b1 = b1.astype(np.float64).reshape(-1)
    W2 = W2.astype(np.float64)                  # [H, 2]
    b2 = b2.astype(np.float64).reshape(-1)

    def gg(v, col):
        return np.tanh(np.outer(v, W1) + b1) @ W2[:, col]

    umax = vmax * vmax
    su = 2.0 / umax

    M = 4000
    wn = np.cos(np.pi * (np.arange(M) + 0.5) / M)
    u = (wn + 1.0) / 2.0 * umax
    v = np.sqrt(np.maximum(u, 1e-12))
    gp = gg(v, 0)
    gm = gg(-v, 0)
    E = (gp + gm) / 2.0          # even part of g0 (== 0 when b1 == 0)
    O = (gp - gm) / 2.0 / v      # odd part / v, a function of u (hence w)

    eb = b2[0] + float(np.mean(E))
    weight = v + 0.02

    # stiction limit, linearized at 0:  L(v) ~ L0 (constant on |v|<0.01)
    g10 = gg(np.array([0.0]), 1)[0] + b2[1]
    L0 = _softplus(g10)

    av = np.abs(v2).astype(np.float64)
    vv = v2.astype(np.float64)
    kin_exact = _softplus(gg(vv, 0) + b2[0])
    is_kin = av >= KARNOPP_DV

    import numpy.polynomial.chebyshev as Cc
    import numpy.polynomial.polynomial as Pp

    wv = (vv * vv) * su - 1.0
    best = None
    for deg in range(3, 13):
        cc = Cc.chebfit(wn, O, deg, w=weight)
        mono = Cc.cheb2poly(cc)
        approx = vv * Pp.polyval(wv, mono)       # ~ g0(v2)
        kin_fit = _softplus(approx + eb)
        err = np.abs((kin_fit - kin_exact)[is_kin])
        l2 = float(np.sqrt(np.sum(err * err)))   # abs L2 over kinetic elems
        if l2 / 13000.0 < 2.5e-3 or deg == 12:
            best = (mono, deg, l2)
            break
    mono, deg, l2 = best

    # roots in w, mapped to u-space: w = su*u - 1  ->  u_r = (w_r + 1)/su
    lead_w = mono[-1]
    roots_w = np.roots(mono[::-1])
    roots_u = (roots_w + 1.0) / su
    # P(w) = lead_w * prod (w - w_r) = lead_w * su^deg * prod (u - u_r)
    lead = lead_w * (su ** deg)

    quads = []   # (b, c): monic u'^2 + b*u' + c
    lins = []
    used = np.zeros(len(roots_u), bool)
    for i, r in enumerate(roots_u):
        if used[i]:
            continue
        if abs(r.imag) > 1e-10 * max(1.0, abs(r)):
            for j in range(i + 1, len(roots_u)):
                if not used[j] and abs(roots_u[j] - r.conjugate()) < 1e-6 * max(1.0, abs(r)):
                    used[j] = True
                    break
            quads.append((-2.0 * r.real, r.real ** 2 + r.imag ** 2))
            used[i] = True
        else:
            used[i] = True
            lins.append(r.real)
    lins.sort()
    while len(lins) >= 2:
        r1 = lins.pop(0)
        r2 = lins.pop(-1)
        quads.append((-(r1 + r2), r1 * r2))
    lin = lins[0] if lins else None

    # overflow guard for the fp16 product chain: rescale u' = s*u
    s = 1.0
    for _ in range(60):
        ug = np.linspace(0.0, umax, 2001) * s
        prod = np.ones_like(ug)
        mx = 0.0
        for (qb, qc) in quads:
            fv = ug * ug + (qb * s) * ug + qc * s * s
            mx = max(mx, np.abs(fv).max())
            prod = prod * fv
            mx = max(mx, np.abs(prod).max())
        if lin is not None:
            prod = prod * (ug - lin * s)
            mx = max(mx, np.abs(prod).max())
        if mx * vmax > 2.0e4 or mx < 1e-12:
            s *= 0.5
        else:
            break
    c_scale = lead / (s ** deg)

    return dict(
        s=s,
        quads=[(qb * s, qc * s * s) for (qb, qc) in quads],
        lin=(lin * s if lin is not None else None),
        c_scale=float(c_scale), eb=float(eb),
        L0=float(L0), deg=deg, fit_l2=l2,
    )


def _build_program(consts):
    """Build the SPMD Bass program (same on all 8 cores)."""
    import concourse.bacc as bacc
    import concourse.mybir as mybir
    import bass_rust as _bass_rust
    from concourse import tile
    from concourse.tile_rust import add_dep_helper
    from concourse.hw_specs import get_activation_tables

    fp32 = mybir.dt.float32
    fp16 = mybir.dt.float16
    u8 = mybir.dt.uint8
    Alu = mybir.AluOpType
    Act = mybir.ActivationFunctionType

    class _Bacc(bacc.Bacc):
        # Force Exp and Ln to resolve to the combined natural_log_exp_and_others
        # table so the whole kernel needs only two table loads (trig + exp/ln).
        def insert_act_table_loads(self):
            has_activation = any(
                isinstance(i, mybir.InstActivation)
                for b in self.main_func.blocks
                for i in b.instructions
            )
            if not has_activation:
                return
            tables = list(get_activation_tables(self.m.arch).items())
            fixed = []
            for name, funcs in tables:
                if name != "trig_and_small":
                    funcs = funcs - {Act.Sign, Act.Identity, Act.Sin}
                if name != "natural_log_exp_and_others":
                    funcs = funcs - {Act.Exp, Act.Ln}
                fixed.append((name, funcs))
            _bass_rust.insert_act_table_loads(self, fixed)

    c = consts
    neg_pi = float(np.float32(-np.pi))
    nc = _Bacc()

    def reg_const(val):
        v = float(val)
        if (fp32, v) not in nc.const_aps.aps:
            tsr = nc.alloc_sbuf_tensor(
                f"constu-f32-{len(nc.const_aps.aps)}", [128, 1], fp32)
            nc.gpsimd.memset(tsr.ap(), v)
            nc.const_aps.aps[(fp32, v)] = tsr.ap()

    for v in (0.0, neg_pi, c["eb"], 1.0):
        reg_const(v)
    nc.all_engine_barrier()

    zin_d = nc.dram_tensor("zin", [NCH, P, NROW_IN * C], fp16,
                           kind="ExternalInput")
    outs_d = {
        nm: nc.dram_tensor(nm, [N_CORE], fp16, kind="ExternalOutput")
        for nm in ("odx1", "odv1", "odx2", "odv2", "odxc")
    }

    def orib(nm, ch):
        return outs_d[nm][:].rearrange("(p i) -> p i", p=P)[:, ch * C:(ch + 1) * C]

    quads = c["quads"]
    lin = c["lin"]
    sq_scale = float(np.sqrt(c["s"]))

    with tile.TileContext(nc) as tc:
        with tc.tile_pool(name="sb", bufs=1) as pool:
            act_instrs = [[], []]   # [trig-phase, expln-phase]

            def tl(tag, dt=fp16, cols=C):
                return pool.tile([P, cols], dt, tag=tag, name=tag)

            for ch in range(NCH):
                sfx = f"_{ch}"
                L = tl("L" + sfx, cols=NROW_IN * C)
                nc.sync.dma_start(out=L[:], in_=zin_d[ch])
                X1 = L[:, 0 * C:1 * C]; V1 = L[:, 1 * C:2 * C]
                X2 = L[:, 2 * C:3 * C]; V2 = L[:, 3 * C:4 * C]
                XCA = L[:, 4 * C:5 * C]; TH = L[:, 5 * C:6 * C]

                # early pass-through stores (only need the load)
                nc.sync.dma_start(out=orib("odx1", ch), in_=V1)
                nc.sync.dma_start(out=orib("odx2", ch), in_=V2)

                # ---- Act phase 1 (trig_and_small): S, U, U2, SGN ----
                S = tl("S" + sfx)
                i1 = nc.scalar.activation(S[:], TH, Act.Sin, bias=neg_pi, scale=0.5)
                U = tl("U" + sfx)    # u' = s * v2^2
                i2 = nc.scalar.activation(U[:], V2, Act.Square, scale=sq_scale)
                U2 = tl("U2" + sfx)  # u'^2
                i3 = nc.scalar.activation(U2[:], U[:], Act.Square)
                SGN = tl("SGN" + sfx)
                i4 = nc.scalar.activation(SGN[:], V2, Act.Sign)
                act_instrs[0] += [i1, i2, i3, i4]

                # ---- DVE: F_net chain (only needs the load) ----
                HA = tl("HA" + sfx)
                nc.vector.tensor_tensor(HA[:], X1, X2, Alu.subtract)
                HB = tl("HB" + sfx)
                nc.vector.tensor_tensor(HB[:], V1, V2, Alu.subtract)
                HBC = tl("HBC" + sfx)
                nc.vector.tensor_single_scalar(HBC[:], HB[:], C2 / K2, Alu.mult)
                B4 = tl("B4" + sfx)
                nc.vector.tensor_tensor(B4[:], HA[:], HBC[:], Alu.add)
                FN = tl("FN" + sfx)   # F_net
                nc.vector.tensor_single_scalar(FN[:], B4[:], K2, Alu.mult)

                # ---- DVE: e, dv1, d_xc ----
                SH = tl("SH" + sfx)   # 0.5 sin(0.5 t)  (S = -sin(0.5t))
                nc.vector.tensor_single_scalar(SH[:], S[:], -0.5, Alu.mult)
                E = tl("E" + sfx)     # x2_ref - x2
                nc.vector.tensor_tensor(E[:], SH[:], X2, Alu.subtract)
                T1 = tl("T1" + sfx)
                nc.vector.tensor_single_scalar(T1[:], E[:], c["K"], Alu.mult)
                A1 = tl("A1" + sfx)   # K*e + A*xc
                nc.vector.tensor_tensor(A1[:], T1[:], XCA, Alu.add)
                T3 = tl("T3" + sfx)
                nc.vector.tensor_single_scalar(T3[:], X1, -K1, Alu.mult)
                T4 = tl("T4" + sfx)
                nc.vector.tensor_single_scalar(T4[:], V1, -C1, Alu.mult)
                A2 = tl("A2" + sfx)
                nc.vector.tensor_tensor(A2[:], T3[:], T4[:], Alu.add)
                A3 = tl("A3" + sfx)
                nc.vector.tensor_tensor(A3[:], A1[:], A2[:], Alu.add)
                ODV1 = tl("ODV1" + sfx)
                nc.vector.tensor_tensor(ODV1[:], A3[:], FN[:], Alu.subtract)
                nc.sync.dma_start(out=orib("odv1", ch), in_=ODV1[:])
                ODXC = tl("ODXC" + sfx)   # e - p*xc = (-p/A)*(A*xc) + e
                nc.vector.scalar_tensor_tensor(ODXC[:], XCA, -c["p"] / c["A"],
                                               E[:], Alu.mult, Alu.add)
                nc.sync.dma_start(out=orib("odxc", ch), in_=ODXC[:])

                # ---- DVE: polynomial for g0 (factored form in u', fp16) ----
                PACC = None
                for qi, (qb, qc) in enumerate(quads):
                    R = tl(f"R{qi}" + sfx)
                    nc.vector.tensor_scalar(R[:], U[:], qb, qc, Alu.mult, Alu.add)
                    FQ = tl(f"FQ{qi}" + sfx)
                    nc.vector.tensor_tensor(FQ[:], U2[:], R[:], Alu.add)
                    if PACC is None:
                        PACC = FQ
                    else:
                        NP_ = tl(f"PP{qi}" + sfx)
                        nc.vector.tensor_tensor(NP_[:], PACC[:], FQ[:], Alu.mult)
                        PACC = NP_
                if lin is not None:
                    FL = tl("FL" + sfx)
                    nc.vector.tensor_scalar(FL[:], U[:], 1.0, -lin,
                                            Alu.mult, Alu.add)
                    if PACC is None:
                        PACC = FL
                    else:
                        NP_ = tl("PPL" + sfx)
                        nc.vector.tensor_tensor(NP_[:], PACC[:], FL[:], Alu.mult)
                        PACC = NP_
                G = tl("G" + sfx)
                nc.vector.tensor_tensor(G[:], PACC[:], V2, Alu.mult)

                # ---- Act phase 2 (natural_log_exp): Q, KIN ----
                Q = tl("Q" + sfx, dt=fp32)
                i5 = nc.scalar.activation(Q[:], G[:], Act.Exp,
                                          bias=c["eb"], scale=c["c_scale"])
                KIN = tl("KIN" + sfx)
                i6 = nc.scalar.activation(KIN[:], Q[:], Act.Ln, bias=1.0)
                act_instrs[1] += [i5, i6]

                # ---- DVE: friction select + dv2 ----
                MASK = pool.tile([P, C], u8, tag="MASK" + sfx, name="MASK" + sfx)
                nc.vector.tensor_single_scalar(MASK[:], U[:],
                                               c["s"] * KARNOPP_DV ** 2, Alu.is_lt)
                MX = tl("MX" + sfx)
                nc.vector.tensor_single_scalar(MX[:], FN[:], -c["L0"], Alu.max)
                MM = tl("MM" + sfx)
                nc.vector.tensor_single_scalar(MM[:], MX[:], c["L0"], Alu.min)
                PSI = tl("PSI" + sfx)   # KIN * sgn(v2)
                nc.vector.tensor_tensor(PSI[:], KIN[:], SGN[:], Alu.mult)
                nc.vector.copy_predicated(PSI[:], MASK[:], MM[:])
                DS = tl("DS" + sfx)
                nc.vector.tensor_tensor(DS[:], FN[:], PSI[:], Alu.subtract)
                ODV2 = tl("ODV2" + sfx)
                nc.vector.tensor_single_scalar(ODV2[:], DS[:], 1.0 / M2, Alu.mult)
                nc.sync.dma_start(out=orib("odv2", ch), in_=ODV2[:])

            # keep the Act engine's table phases coherent: chain nosync deps
            seq = act_instrs[0] + act_instrs[1]
            for a, b in zip(seq, seq[1:]):
                add_dep_helper(b.ins, a.ins, sync=False, reason="act table order")

    nc.finalize()
    return nc


def _prepare(inputs):
    """Host-side constant folding + program build (cached on weight values)."""
    logK = np.float32(inputs["logK"]); logz = np.float32(inputs["logz"])
    logp = np.float32(inputs["logp"])
    W1 = np.asarray(inputs["W1"], dtype=np.float32)
    b1 = np.asarray(inputs["b1"], dtype=np.float32)
    W2 = np.asarray(inputs["W2"], dtype=np.float32)
    b2 = np.asarray(inputs["b2"], dtype=np.float32)
    v2 = np.asarray(inputs["z"][3], dtype=np.float32)
    vmax = float(np.abs(v2).max()) * 1.02 + 1e-3

    key = (logK.tobytes(), logz.tobytes(), logp.tobytes(), W1.tobytes(),
           b1.tobytes(), W2.tobytes(), b2.tobytes(), round(vmax, 3))
    if key in _compile_cache:
        return _compile_cache[key]

    K = np.float32(np.exp(logK))
    z_ctrl = np.float32(np.exp(logz))
    p_ctrl = np.float32(np.exp(logp))
    A = np.float32(K * (z_ctrl - p_ctrl))

    fit = _fit_friction(W1, b1, W2, b2, v2, vmax)

    consts = dict(
        K=float(K), p=float(p_ctrl), A=float(A),
        s=fit["s"], quads=fit["quads"], lin=fit["lin"],
        c_scale=fit["c_scale"], eb=fit["eb"], L0=fit["L0"],
    )
    nc = _build_program(consts)
    _compile_cache[key] = (nc, fit, consts)
    return nc, fit, consts


def _run(inputs, trace=False):
    from concourse.bass_utils import run_bass_kernel_spmd

    nc, _fit, consts = _prepare(inputs)

    t = np.ascontiguousarray(np.asarray(inputs["t"], dtype=np.float32))
    z = np.ascontiguousarray(np.asarray(inputs["z"], dtype=np.float32))

    # pack [6, N]: x1, v1, x2, v2, A*xc, t  -> fp16 [cores, NCH, P, 6, C]
    rows = np.empty((NROW_IN, N_TOTAL), dtype=np.float16)
    rows[0] = z[0]; rows[1] = z[1]; rows[2] = z[2]; rows[3] = z[3]
    rows[4] = (z[4].astype(np.float64) * consts["A"]).astype(np.float16)
    rows[5] = t
    pk = rows.reshape(NROW_IN, N_CORES, P, NCH, C).transpose(1, 3, 2, 0, 4)
    pk = np.ascontiguousarray(pk)  # [cores, NCH, P, 6, C]

    in_maps = [{"zin": pk[i].reshape(NCH, P, NROW_IN * C)} for i in range(N_CORES)]

    res = run_bass_kernel_spmd(nc, in_maps, core_ids=list(range(N_CORES)),
                               trace=trace)
    out = np.empty((5, N_TOTAL), dtype=np.float32)
    names = ["odx1", "odv1", "odx2", "odv2", "odxc"]
    for i in range(N_CORES):
        sl = slice(i * N_CORE, (i + 1) * N_CORE)
        for r, nm in enumerate(names):
            out[r, sl] = res.results[i][nm].reshape(N_CORE)
    return out, res


def kernel(**inputs):
    out, _res = _run(inputs, trace=False)
    return out


# revision 4
# speedup vs baseline: 2.4293x; 1.0699x over previous
"""Trainium2 Bass kernel: batched controlled-system dynamics (N = 2^20 states).

Strategy (v4):
  - Pure data parallel over 8 NeuronCores: contiguous slices of the batch axis.
  - Everything elementwise in a ribbon layout [128, 1024] per core, fp16
    tensors, two column-chunks for load/compute/store overlap.
  - Host packs ALL inputs into one fp16 array [NCH, 128, 6, C]
    (x1, v1, x2, v2, A*xc, t): ONE load DMA per chunk, 6KB partition lines.
    dv1/dv2/dxc are packed into ONE fp16 store per chunk; dx1/dx2 ship
    straight from the input views.  Host unpacks + upcasts to fp32.
    Total HBM traffic ~2.75MB/core, 8 DMAs.
  - Friction MLP (1 -> 64 -> 2, tanh + softplus heads) collapses to 1D
    functions of v2 (b1 == 0 makes g0 odd):
      kinetic = softplus(lead * G + eb),  G = v2 * prod_j f_j(u'), u' = s*v2^2
    with monic quadratic/linear factors f_j of the Chebyshev fit, evaluated
    in fp16 on DVE; `lead` folds into the Exp activation's input scale and
    the overflow-guard rescale s into the Square activation's input scale.
    Stiction limit uses a CONSTANT bound L0 (the |v2| < 0.01 window makes
    the L1*v2 term negligible).
  - Emission order front-loads both chunks' polynomial chains so the Act
    engine's exp/ln phase starts early; dv1/dxc trees fill DVE while the
    friction selects wait on KIN.
  - Engine split: Act does sin/squares/sign/exp/ln plus three affine
    Identities; DVE does the rest.  Pool is never used for compute.
"""

import numpy as np

# physical system constants (match the reference)
M1, M2 = 1.0, 1.5
K1, K2 = 2.0, 3.0
C1, C2 = 0.5, 0.8
KARNOPP_DV = 0.01
REF_AMP, REF_OMEGA = 0.5, 0.5

N_CORES = 8
N_TOTAL = 1 << 20
N_CORE = N_TOTAL // N_CORES    # 131072
P = 128
F = N_CORE // P                # 1024 ribbon columns
NCH = 2                        # compute/DMA chunks
C = F // NCH                   # 512 columns per chunk
NROW_IN = 6                    # x1 v1 x2 v2 xc' t
NROW_OUT = 3                   # dv1 dv2 dxc

_compile_cache = {}


def _softplus(x):
    return np.log1p(np.exp(-np.abs(x))) + np.maximum(x, 0.0)


def _fit_friction(W1, b1, W2, b2, v2, vmax):
    """Collapse the friction MLP to 1D functions of v2 and pick the cheapest
    factored-polynomial representation whose exact, data-weighted L2 error
    stays within budget."""
    W1 = W1.astype(np.float64).reshape(-1)      # [H]
    b1 = b1.astype(np.float64).reshape(-1)
    W2 = W2.astype(np.float64)                  # [H, 2]
    b2 = b2.astype(np.float64).reshape(-1)

    def gg(v, col):
        return np.tanh(np.outer(v, W1) + b1) @ W2[:, col]

    umax = vmax * vmax
    su = 2.0 / umax

    M = 4000
    wn = np.cos(np.pi * (np.arange(M) + 0.5) / M)
    u = (wn + 1.0) / 2.0 * umax
    v = np.sqrt(np.maximum(u, 1e-12))
    gp = gg(v, 0)
    gm = gg(-v, 0)
    E = (gp + gm) / 2.0          # even part of g0 (== 0 when b1 == 0)
    O = (gp - gm) / 2.0 / v      # odd part / v, a function of u (hence w)

    eb = b2[0] + float(np.mean(E))
    weight = v + 0.02

    # stiction limit: constant L0 on the |v| < 0.01 static window
    g10 = gg(np.array([0.0]), 1)[0] + b2[1]
    L0 = _softplus(g10)

    av = np.abs(v2).astype(np.float64)
    vv = v2.astype(np.float64)
    kin_exact = _softplus(gg(vv, 0) + b2[0])
    is_kin = av >= KARNOPP_DV

    import numpy.polynomial.chebyshev as Cc
    import numpy.polynomial.polynomial as Pp

    wv = (vv * vv) * su - 1.0
    best = None
    for deg in range(3, 13):
        cc = Cc.chebfit(wn, O, deg, w=weight)
        mono = Cc.cheb2poly(cc)
        approx = vv * Pp.polyval(wv, mono)       # ~ g0(v2)
        kin_fit = _softplus(approx + eb)
        err = np.abs((kin_fit - kin_exact)[is_kin])
        l2 = float(np.sqrt(np.sum(err * err)))   # abs L2 over kinetic elems
        if l2 / 13000.0 < 2.5e-3 or deg == 12:
            best = (mono, deg, l2)
            break
    mono, deg, l2 = best

    # roots in w, mapped to u-space: w = su*u - 1  ->  u_r = (w_r + 1)/su
    lead_w = mono[-1]
    roots_w = np.roots(mono[::-1])
    roots_u = (roots_w + 1.0) / su
    lead = lead_w * (su ** deg)

    quads = []   # (b, c): monic u'^2 + b*u' + c
    lins = []
    used = np.zeros(len(roots_u), bool)
    for i, r in enumerate(roots_u):
        if used[i]:
            continue
        if abs(r.imag) > 1e-10 * max(1.0, abs(r)):
            for j in range(i + 1, len(roots_u)):
                if not used[j] and abs(roots_u[j] - r.conjugate()) < 1e-6 * max(1.0, abs(r)):
                    used[j] = True
                    break
            quads.append((-2.0 * r.real, r.real ** 2 + r.imag ** 2))
            used[i] = True
        else:
            used[i] = True
            lins.append(r.real)
    lins.sort()
    while len(lins) >= 2:
        r1 = lins.pop(0)
        r2 = lins.pop(-1)
        quads.append((-(r1 + r2), r1 * r2))
    lin = lins[0] if lins else None

    # overflow guard for the fp16 product chain: rescale u' = s*u
    s = 1.0
    for _ in range(60):
        ug = np.linspace(0.0, umax, 2001) * s
        prod = np.ones_like(ug)
        mx = 0.0
        for (qb, qc) in quads:
            fv = ug * ug + (qb * s) * ug + qc * s * s
            mx = max(mx, np.abs(fv).max())
            prod = prod * fv
            mx = max(mx, np.abs(prod).max())
        if lin is not None:
            prod = prod * (ug - lin * s)
            mx = max(mx, np.abs(prod).max())
        if mx * vmax > 2.0e4 or mx < 1e-12:
            s *= 0.5
        else:
            break
    c_scale = lead / (s ** deg)

    return dict(
        s=s,
        quads=[(qb * s, qc * s * s) for (qb, qc) in quads],
        lin=(lin * s if lin is not None else None),
        c_scale=float(c_scale), eb=float(eb),
        L0=float(L0), deg=deg, fit_l2=l2,
    )


def _build_program(consts):
    """Build the SPMD Bass program (same on all 8 cores)."""
    import concourse.bacc as bacc
    import concourse.mybir as mybir
    import bass_rust as _bass_rust
    from concourse import tile
    from concourse.tile_rust import add_dep_helper
    from concourse.hw_specs import get_activation_tables

    fp32 = mybir.dt.float32
    fp16 = mybir.dt.float16
    u8 = mybir.dt.uint8
    Alu = mybir.AluOpType
    Act = mybir.ActivationFunctionType

    class _Bacc(bacc.Bacc):
        # Force Exp and Ln to resolve to the combined natural_log_exp_and_others
        # table so the whole kernel needs only two table loads (trig + exp/ln).
        def insert_act_table_loads(self):
            has_activation = any(
                isinstance(i, mybir.InstActivation)
                for b in self.main_func.blocks
                for i in b.instructions
            )
            if not has_activation:
                return
            tables = list(get_activation_tables(self.m.arch).items())
            fixed = []
            for name, funcs in tables:
                if name != "trig_and_small":
                    funcs = funcs - {Act.Sign, Act.Identity, Act.Sin}
                if name != "natural_log_exp_and_others":
                    funcs = funcs - {Act.Exp, Act.Ln}
                fixed.append((name, funcs))
            _bass_rust.insert_act_table_loads(self, fixed)

    c = consts
    neg_pi = float(np.float32(-np.pi))
    nc = _Bacc()

    def reg_const(val):
        v = float(val)
        if (fp32, v) not in nc.const_aps.aps:
            tsr = nc.alloc_sbuf_tensor(
                f"constu-f32-{len(nc.const_aps.aps)}", [128, 1], fp32)
            nc.gpsimd.memset(tsr.ap(), v)
            nc.const_aps.aps[(fp32, v)] = tsr.ap()

    for v in (0.0, neg_pi, c["eb"], 1.0):
        reg_const(v)
    nc.all_engine_barrier()

    zin_d = nc.dram_tensor("zin", [NCH, P, NROW_IN * C], fp16,
                           kind="ExternalInput")
    om_d = nc.dram_tensor("om", [NCH, P, NROW_OUT * C], fp16,
                          kind="ExternalOutput")
    ox1_d = nc.dram_tensor("ox1", [N_CORE], fp16, kind="ExternalOutput")
    ox2_d = nc.dram_tensor("ox2", [N_CORE], fp16, kind="ExternalOutput")

    quads = c["quads"]
    lin = c["lin"]
    sq_scale = float(np.sqrt(c["s"]))

    with tile.TileContext(nc) as tc:
        with tc.tile_pool(name="sb", bufs=1) as pool:
            act_seq = []

            def tl(tag, dt=fp16, cols=C):
                return pool.tile([P, cols], dt, tag=tag, name=tag)

            # per-chunk state dicts
            st = [dict() for _ in range(NCH)]

            # ---- loads + pass-through stores ----
            for ch in range(NCH):
                d = st[ch]
                L = tl(f"L_{ch}", cols=NROW_IN * C)
                nc.sync.dma_start(out=L[:], in_=zin_d[ch])
                d["X1"] = L[:, 0 * C:1 * C]; d["V1"] = L[:, 1 * C:2 * C]
                d["X2"] = L[:, 2 * C:3 * C]; d["V2"] = L[:, 3 * C:4 * C]
                d["XCA"] = L[:, 4 * C:5 * C]; d["TH"] = L[:, 5 * C:6 * C]
                nc.sync.dma_start(
                    out=ox1_d[:].rearrange("(p i) -> p i", p=P)[:, ch * C:(ch + 1) * C],
                    in_=d["V1"])
                nc.sync.dma_start(
                    out=ox2_d[:].rearrange("(p i) -> p i", p=P)[:, ch * C:(ch + 1) * C],
                    in_=d["V2"])

            # ---- Act phase 1 (trig table): S, U, U2, SGN, SH, T3, T4 ----
            for ch in range(NCH):
                d = st[ch]; sfx = f"_{ch}"
                S = tl("S" + sfx)
                act_seq.append(nc.scalar.activation(S[:], d["TH"], Act.Sin,
                                                    bias=neg_pi, scale=0.5))
                U = tl("U" + sfx)    # u' = s * v2^2
                act_seq.append(nc.scalar.activation(U[:], d["V2"], Act.Square,
                                                    scale=sq_scale))
                U2 = tl("U2" + sfx)  # u'^2
                act_seq.append(nc.scalar.activation(U2[:], U[:], Act.Square))
                SGN = tl("SGN" + sfx)
                act_seq.append(nc.scalar.activation(SGN[:], d["V2"], Act.Sign))
                SH = tl("SH" + sfx)  # 0.5 sin(0.5 t)   (S = -sin(0.5t))
                act_seq.append(nc.scalar.activation(SH[:], S[:], Act.Identity,
                                                    scale=-0.5))
                T3 = tl("T3" + sfx)
                act_seq.append(nc.scalar.activation(T3[:], d["X1"], Act.Identity,
                                                    scale=-K1))
                T4 = tl("T4" + sfx)
                act_seq.append(nc.scalar.activation(T4[:], d["V1"], Act.Identity,
                                                    scale=-C1))
                d.update(S=S, U=U, U2=U2, SGN=SGN, SH=SH, T3=T3, T4=T4)

            # ---- DVE: F_net chains + polynomial chains (front-loaded) ----
            for ch in range(NCH):
                d = st[ch]; sfx = f"_{ch}"
                HA = tl("HA" + sfx)
                nc.vector.tensor_tensor(HA[:], d["X1"], d["X2"], Alu.subtract)
                HB = tl("HB" + sfx)
                nc.vector.tensor_tensor(HB[:], d["V1"], d["V2"], Alu.subtract)
                HBC = tl("HBC" + sfx)
                nc.vector.tensor_single_scalar(HBC[:], HB[:], C2 / K2, Alu.mult)
                B4 = tl("B4" + sfx)
                nc.vector.tensor_tensor(B4[:], HA[:], HBC[:], Alu.add)
                FN = tl("FN" + sfx)   # F_net
                nc.vector.tensor_single_scalar(FN[:], B4[:], K2, Alu.mult)
                d["FN"] = FN

                # polynomial for g0 (factored form in u', fp16)
                U, U2 = d["U"], d["U2"]
                PACC = None
                for qi, (qb, qc) in enumerate(quads):
                    R = tl(f"R{qi}" + sfx)
                    nc.vector.tensor_scalar(R[:], U[:], qb, qc, Alu.mult, Alu.add)
                    FQ = tl(f"FQ{qi}" + sfx)
                    nc.vector.tensor_tensor(FQ[:], U2[:], R[:], Alu.add)
                    if PACC is None:
                        PACC = FQ
                    else:
                        NP_ = tl(f"PP{qi}" + sfx)
                        nc.vector.tensor_tensor(NP_[:], PACC[:], FQ[:], Alu.mult)
                        PACC = NP_
                if lin is not None:
                    FL = tl("FL" + sfx)
                    nc.vector.tensor_scalar(FL[:], U[:], 1.0, -lin,
                                            Alu.mult, Alu.add)
                    if PACC is None:
                        PACC = FL
                    else:
                        NP_ = tl("PPL" + sfx)
                        nc.vector.tensor_tensor(NP_[:], PACC[:], FL[:], Alu.mult)
                        PACC = NP_
                G = tl("G" + sfx)
                nc.vector.tensor_tensor(G[:], PACC[:], d["V2"], Alu.mult)
                d["G"] = G

            # ---- Act phase 2 (natural_log_exp): Q, KIN ----
            for ch in range(NCH):
                d = st[ch]; sfx = f"_{ch}"
                Q = tl("Q" + sfx, dt=fp32)
                act_seq.append(nc.scalar.activation(Q[:], d["G"][:], Act.Exp,
                                                    bias=c["eb"], scale=c["c_scale"]))
                KIN = tl("KIN" + sfx)
                act_seq.append(nc.scalar.activation(KIN[:], Q[:], Act.Ln, bias=1.0))
                d["KIN"] = KIN

            # ---- DVE: select-prep + trees + selects, chunk-major ----
            for ch in range(NCH):
                d = st[ch]; sfx = f"_{ch}"
                OUT = tl("OUT" + sfx, cols=NROW_OUT * C)
                ODV1 = OUT[:, 0 * C:1 * C]
                ODV2 = OUT[:, 1 * C:2 * C]
                ODXC = OUT[:, 2 * C:3 * C]
                FN = d["FN"]

                MASK = pool.tile([P, C], u8, tag="MASK" + sfx, name="MASK" + sfx)
                nc.vector.tensor_single_scalar(MASK[:], d["U"][:],
                                               c["s"] * KARNOPP_DV ** 2, Alu.is_lt)
                MX = tl("MX" + sfx)
                nc.vector.tensor_single_scalar(MX[:], FN[:], -c["L0"], Alu.max)
                MM = tl("MM" + sfx)
                nc.vector.tensor_single_scalar(MM[:], MX[:], c["L0"], Alu.min)

                E = tl("E" + sfx)     # x2_ref - x2
                nc.vector.tensor_tensor(E[:], d["SH"][:], d["X2"], Alu.subtract)
                T1 = tl("T1" + sfx)
                nc.vector.tensor_single_scalar(T1[:], E[:], c["K"], Alu.mult)
                A1 = tl("A1" + sfx)   # K*e + A*xc
                nc.vector.tensor_tensor(A1[:], T1[:], d["XCA"], Alu.add)
                A2 = tl("A2" + sfx)
                nc.vector.tensor_tensor(A2[:], d["T3"][:], d["T4"][:], Alu.add)
                A3 = tl("A3" + sfx)
                nc.vector.tensor_tensor(A3[:], A1[:], A2[:], Alu.add)
                nc.vector.tensor_tensor(ODV1, A3[:], FN[:], Alu.subtract)
                # d_xc = e - p*xc = (-p/A)*(A*xc) + e
                nc.vector.scalar_tensor_tensor(ODXC, d["XCA"], -c["p"] / c["A"],
                                               E[:], Alu.mult, Alu.add)

                PSI = tl("PSI" + sfx)   # KIN * sgn(v2)
                nc.vector.tensor_tensor(PSI[:], d["KIN"][:], d["SGN"][:], Alu.mult)
                nc.vector.copy_predicated(PSI[:], MASK[:], MM[:])
                DS = tl("DS" + sfx)
                nc.vector.tensor_tensor(DS[:], FN[:], PSI[:], Alu.subtract)
                nc.vector.tensor_single_scalar(ODV2, DS[:], 1.0 / M2, Alu.mult)

                nc.sync.dma_start(out=om_d[ch], in_=OUT[:])

            # keep the Act engine's table phases coherent: chain nosync deps
            for a, b in zip(act_seq, act_seq[1:]):
                add_dep_helper(b.ins, a.ins, sync=False, reason="act table order")

    nc.finalize()
    return nc


def _prepare(inputs):
    """Host-side constant folding + program build (cached on weight values)."""
    logK = np.float32(inputs["logK"]); logz = np.float32(inputs["logz"])
    logp = np.float32(inputs["logp"])
    W1 = np.asarray(inputs["W1"], dtype=np.float32)
    b1 = np.asarray(inputs["b1"], dtype=np.float32)
    W2 = np.asarray(inputs["W2"], dtype=np.float32)
    b2 = np.asarray(inputs["b2"], dtype=np.float32)
    v2 = np.asarray(inputs["z"][3], dtype=np.float32)
    vmax = float(np.abs(v2).max()) * 1.02 + 1e-3

    key = (logK.tobytes(), logz.tobytes(), logp.tobytes(), W1.tobytes(),
           b1.tobytes(), W2.tobytes(), b2.tobytes(), round(vmax, 3))
    if key in _compile_cache:
        return _compile_cache[key]

    K = np.float32(np.exp(logK))
    z_ctrl = np.float32(np.exp(logz))
    p_ctrl = np.float32(np.exp(logp))
    A = np.float32(K * (z_ctrl - p_ctrl))

    fit = _fit_friction(W1, b1, W2, b2, v2, vmax)

    consts = dict(
        K=float(K), p=float(p_ctrl), A=float(A),
        s=fit["s"], quads=fit["quads"], lin=fit["lin"],
        c_scale=fit["c_scale"], eb=fit["eb"], L0=fit["L0"],
    )
    nc = _build_program(consts)
    _compile_cache[key] = (nc, fit, consts)
    return nc, fit, consts


def _run(inputs, trace=False):
    from concourse.bass_utils import run_bass_kernel_spmd

    nc, _fit, consts = _prepare(inputs)

    t = np.ascontiguousarray(np.asarray(inputs["t"], dtype=np.float32))
    z = np.ascontiguousarray(np.asarray(inputs["z"], dtype=np.float32))

    # pack [6, N]: x1, v1, x2, v2, A*xc, t  -> fp16 [cores, NCH, P, 6, C]
    rows = np.empty((NROW_IN, N_TOTAL), dtype=np.float16)
    rows[0] = z[0]; rows[1] = z[1]; rows[2] = z[2]; rows[3] = z[3]
    rows[4] = (z[4].astype(np.float64) * consts["A"]).astype(np.float16)
    rows[5] = t
    pk = rows.reshape(NROW_IN, N_CORES, P, NCH, C).transpose(1, 3, 2, 0, 4)
    pk = np.ascontiguousarray(pk)  # [cores, NCH, P, 6, C]

    in_maps = [{"zin": pk[i].reshape(NCH, P, NROW_IN * C)} for i in range(N_CORES)]

    res = run_bass_kernel_spmd(nc, in_maps, core_ids=list(range(N_CORES)),
                               trace=trace)
    out = np.empty((5, N_TOTAL), dtype=np.float32)
    for i in range(N_CORES):
        sl = slice(i * N_CORE, (i + 1) * N_CORE)
        om = res.results[i]["om"].reshape(NCH, P, NROW_OUT, C)
        om = om.transpose(2, 1, 0, 3).reshape(NROW_OUT, N_CORE)
        out[0, sl] = res.results[i]["ox1"].reshape(N_CORE)
        out[1, sl] = om[0]
        out[2, sl] = res.results[i]["ox2"].reshape(N_CORE)
        out[3, sl] = om[1]
        out[4, sl] = om[2]
    return out, res


def kernel(**inputs):
    out, _res = _run(inputs, trace=False)
    return out
